# revision 80
# baseline (speedup 1.0000x reference)
"""Trainium2 Bass kernel for nn_Attention_22299470201527.

Dense transformer attention block:
  LayerNorm -> Wq/Wkv projections -> per-head QK RMSNorm -> 2D RoPE ->
  softmax(QK^T) V -> Wo projection,  B=8, N=1024, DIM=1024, H=16, DH=64.

Sharding: data-parallel over batch — 8 batch elements on 8 NeuronCores,
one per core, weights replicated, no collectives. kernel(**inputs) takes
the full unsharded inputs and returns the full [8, 1024, 1024] output.

Design notes (~313us HW exec; pair-granular predecessor 341us, f32r
baseline 397us):
  * All matmul operands fp16/bf16 (FWL weight loads, half the DMA bytes).
  * Weights/trig tables are relaid out host-side to partition-major so
    every DMA lands as 128 large contiguous descriptors.
  * x tiles are prefetched ahead of the weight streams; each weight
    matrix streams during the previous projection loop (wpool bufs=2).
  * LayerNorm is fused into the Q loop; LN math runs two iterations
    ahead, xn transposes one ahead of proj, and the RoPE-output PE
    transposes three behind, so the in-order PE queue rarely blocks on
    the eviction -> square -> reduce -> sqrt -> RoPE -> rinv chain.
  * Attention runs at HEAD granularity: one [128 keys, 1024 q] dots
    psum tile (2 banks) per (head, key-tile) step, double-buffered, so
    QK of step s+1 overlaps exp of step s and the ACT engine streams
    exp back-to-back at ~1.05us/tile — ACT is the phase-C roofline
    (128 exps ~ 138us).  PSUM: dots 2x2 banks + AV accumulators 4x1.
  * AV for head h-1 rides along with QK/exp of head h; the last head's
    AV tail runs on fresh psum banks (dots pool closed) to avoid WAR
    waits and a p-state dip.
  * Softmax denominators: the V stationary is [dh(64) | ones(64)], so
    AV psum rows 64-127 come out as 64 pre-broadcast copies of the
    denominator.  Normalization = one bf16 psum eviction (frees the
    bank fast) + reciprocal_approx_fast + multiply, all on DVE, with
    fp16 results written into the dead xnT buffer for Wo.  (No DRAM
    bounce, no partition broadcast: approx_fast only reads base-0
    SBUF — it silently returns garbage on PSUM or partition-64 APs.)
  * Heads 0-2 pre-compute QK/exp inside the V-projection loop (3 exp
    steps per 3.4us projection block keeps ACT fed), with av(0) riding
    the back half on its own psum bank pair (V projections run as
    1-bank halves to make room), so phase C starts with zero backlog.
  * Wo accumulates per token tile with a 3-deep psum pipeline; output
    eviction splits ACT/DVE halves, one 512KB DMA per tile.
"""

import sys

for _p in ("/opt/trn_rl_repo",):
    if _p not in sys.path:
        sys.path.append(_p)

import concourse.bacc as bacc
import concourse.bass as bass
import concourse.tile as tile
from concourse import mybir

F32 = mybir.dt.float32
F16 = mybir.dt.float16
BF16 = mybir.dt.bfloat16

B, N, DIM, H, DH = 8, 1024, 1024, 16, 64
INNER = H * DH
KT = DIM // 128
MT = N // 128
FT = INNER // 128
EPS_LN = 1e-5
EPS_NORM = 1e-12


def _bcast_heads(ap2d, nheads=H):
    """[128, D] AP -> [128, nheads, D], stride-0 broadcast over heads."""
    return bass.AP(
        tensor=ap2d.tensor, offset=ap2d.offset,
        ap=[ap2d.ap[0], [0, nheads], ap2d.ap[1]],
    )


def _bcast_last(ap2d, n):
    """[128, Hn] AP -> [128, Hn, n], stride-0 broadcast innermost."""
    return bass.AP(
        tensor=ap2d.tensor, offset=ap2d.offset,
        ap=[ap2d.ap[0], ap2d.ap[1], [0, n]],
    )


def _rot_view(tile_ap):
    """[128, 1024] tile viewed [128, H, 2, 2, 16] with adjacent 16-blocks
    swapped (rotate-half shuffle; signs live in the sin table)."""
    return bass.AP(
        tensor=tile_ap.tensor, offset=tile_ap.offset + 16,
        ap=[tile_ap.ap[0], [DH, H], [32, 2], [-16, 2], [1, 16]],
    )


def build_nc(has_bias: bool):
    nc = bacc.Bacc("TRN2", target_bir_lowering=False, debug=False, num_devices=8)

    x_d = nc.dram_tensor("x", [N, DIM], F16, kind="ExternalInput")
    # weights partition-major: [128, KT, INNER] flattened per partition
    wq_d = nc.dram_tensor("wq", [128, KT * INNER], F16, kind="ExternalInput")
    wk_d = nc.dram_tensor("wk", [128, KT * INNER], F16, kind="ExternalInput")
    wv_d = nc.dram_tensor("wv", [128, KT * INNER], F16, kind="ExternalInput")
    wo_d = nc.dram_tensor("wo", [128, KT * INNER], F16, kind="ExternalInput")
    id16_d = nc.dram_tensor("ident16", [128, 128], F16, kind="ExternalInput")
    cos_d = nc.dram_tensor("cos_t", [128, MT * DH], F16, kind="ExternalInput")
    sin_d = nc.dram_tensor("sin_t", [128, MT * DH], F16, kind="ExternalInput")
    if has_bias:
        bq_d = nc.dram_tensor("bq", [1, INNER], F16, kind="ExternalInput")
        bkv_d = nc.dram_tensor("bkv", [1, 2 * INNER], F16, kind="ExternalInput")
    out_d = nc.dram_tensor("out", [N, DIM], F32, kind="ExternalOutput")

    with tile.TileContext(nc) as tc:
        with (
            tc.tile_pool(name="const", bufs=1) as constp,
            tc.tile_pool(name="wpool", bufs=2) as wpool,
            tc.tile_pool(name="stats", bufs=2) as stats,
            tc.tile_pool(name="bc", bufs=1) as bc,
            tc.tile_pool(name="xa", bufs=1) as xa,
        ):
            # Prefetch first x tiles BEFORE everything else so LayerNorm
            # starts immediately.
            XPRE = 2
            x_tiles = {}

            def x_fetch(m):
                x_t = xa.tile([128, DIM], F16, tag="x", bufs=2, name=f"x{m%2}")
                nc.sync.dma_start(x_t[:], x_d[m * 128:(m + 1) * 128, :])
                x_tiles[m] = x_t

            for m in range(XPRE):
                x_fetch(m)

            ident_h = constp.tile([128, 128], F16)
            nc.sync.dma_start(ident_h[:], id16_d[:])
            eps_t = constp.tile([128, 1], F32)
            nc.vector.memset(eps_t[:], EPS_LN)

            def stream_w(dram_t):
                w = wpool.tile([128, KT, INNER], F16, tag="w")
                nc.sync.dma_start(
                    w[:], dram_t[:].rearrange("p (a i) -> p a i", a=KT)
                )
                return w

            w_q = stream_w(wq_d)

            cos_sb = constp.tile([128, MT, DH], F16)
            sin_sb = constp.tile([128, MT, DH], F16)
            nc.sync.dma_start(cos_sb[:], cos_d[:].rearrange("p (a d) -> p a d", a=MT))
            nc.sync.dma_start(sin_sb[:], sin_d[:].rearrange("p (a d) -> p a d", a=MT))
            bq_sb = bkv_sb = ones1 = None
            if has_bias:
                bq_sb = constp.tile([1, INNER], F16)
                bkv_sb = constp.tile([1, 2 * INNER], F16)
                nc.sync.dma_start(bq_sb[:], bq_d[:])
                nc.sync.dma_start(bkv_sb[:], bkv_d[:])
                ones1 = constp.tile([1, 128], F16)
                nc.vector.memset(ones1[:], 1.0)

            # Long-lived activations.
            qT = bc.tile([128, FT, N], F16)
            kT = bc.tile([128, FT, N], F16)
            xnT = bc.tile([128, KT, N], F16)   # reused as outT in phase C/D
            # V stationary is [dh (64) | ones (64)] per head: AV psum rows
            # 64-127 come out as 64 pre-broadcast copies of the softmax
            # denominator, so normalization needs no partition broadcast.
            v_sb = bc.tile([128, MT, H, 2 * DH], BF16)
            nc.gpsimd.memset(
                bass.AP(
                    tensor=v_sb.tensor, offset=v_sb[:].offset + DH,
                    ap=[v_sb[:].ap[0], [H * 2 * DH, MT], [2 * DH, H],
                        [1, DH]],
                ),
                1.0,
            )

            ep_cm = tc.tile_pool(name="ep", bufs=1)
            ep = ep_cm.__enter__()
            cp_cm = tc.tile_pool(name="cpool", bufs=1)
            cpool = cp_cm.__enter__()

            with tc.tile_pool(name="pb", bufs=1) as pb:
                pp_cm = tc.tile_pool(name="pp", bufs=2, space="PSUM")
                pp = pp_cm.__enter__()
                tp_cm = tc.tile_pool(name="tp", bufs=4, space="PSUM")
                tp = tp_cm.__enter__()
                def ln_math(m):
                    """LayerNorm stats + normalized fp16 tile for x tile m."""
                    x_t = x_tiles.pop(m)
                    st = stats.tile([128, 2, 6], F32, tag="bst")
                    for g in range(2):
                        nc.vector.bn_stats(st[:, g, :], x_t[:, g * 512:(g + 1) * 512])
                    mv = stats.tile([128, 2], F32, tag="mv")
                    nc.vector.bn_aggr(mv[:], st[:])
                    sd = stats.tile([128, 1], F32, tag="sd")
                    nc.scalar.activation(
                        sd[:], mv[:, 1:2], mybir.ActivationFunctionType.Sqrt,
                        bias=eps_t[:], scale=1.0,
                    )
                    rstd = stats.tile([128, 1], F32, tag="rstd")
                    nc.vector.reciprocal(rstd[:], sd[:])
                    nmu = stats.tile([128, 1], F32, tag="nmu")
                    nc.vector.scalar_tensor_tensor(
                        out=nmu[:], in0=mv[:, 0:1], scalar=-1.0, in1=rstd[:],
                        op0=mybir.AluOpType.mult, op1=mybir.AluOpType.mult,
                    )
                    xn_t = xa.tile([128, DIM], F16, tag="xn", bufs=3)
                    nc.scalar.activation(
                        xn_t[:], x_t[:], mybir.ActivationFunctionType.Identity,
                        bias=nmu[:], scale=rstd[:],
                    )
                    if m + XPRE < MT:
                        x_fetch(m + XPRE)
                    return xn_t

                def ln_transp(xn_t, m):
                    for g in range(2):
                        tps = tp.tile([128, 512], F16, tag="tp", bufs=4)
                        for b4 in range(4):
                            k = g * 4 + b4
                            nc.tensor.transpose(
                                tps[:, b4 * 128:(b4 + 1) * 128],
                                xn_t[:, k * 128:(k + 1) * 128],
                                ident_h[:],
                            )
                        # split the two evictions across ACT and DVE so the
                        # dependent proj(m) isn't gated by the ACT queue
                        dst = xnT[:, g * 4:(g + 1) * 4, m * 128:(m + 1) * 128]
                        src = tps[:].rearrange("p (a t) -> p a t", a=4)
                        if g == 0:
                            nc.scalar.copy(dst, src)
                        else:
                            nc.vector.tensor_copy(dst, src)

                def proj(w, m, psp, bias_sb=None, bias_off=0):
                    """One [128, INNER] projection psum tile for token tile m.
                    Matmul moving free dim is capped at 512, so each half is
                    its own accumulation group."""
                    ps = psp.tile([128, INNER], F32, tag="pp", bufs=2)
                    for nh in range(2):
                        sl = slice(nh * 512, (nh + 1) * 512)
                        if bias_sb is not None:
                            nc.tensor.matmul(
                                ps[:, sl], ones1[:],
                                bias_sb[:, bias_off + nh * 512:
                                        bias_off + (nh + 1) * 512],
                                start=True, stop=False,
                            )
                        for k in range(KT):
                            nc.tensor.matmul(
                                ps[:, sl],
                                xnT[:, k, m * 128:(m + 1) * 128],
                                w[:, k, nh * 512:(nh + 1) * 512],
                                start=(k == 0 and bias_sb is None),
                                stop=(k == KT - 1),
                            )
                    return ps

                def rms_rope(ps, m, dve_add=False):
                    qtmp = pb.tile([128, INNER], F16, tag="qtmp", bufs=2)
                    nc.scalar.copy(qtmp[:], ps[:])
                    sq = pb.tile([128, INNER], F16, tag="sq", bufs=1)
                    nc.vector.tensor_mul(sq[:], qtmp[:], qtmp[:])
                    ssq = stats.tile([128, H], F32, tag="ssq")
                    nc.vector.reduce_sum(
                        ssq[:], sq[:].rearrange("p (h d) -> p h d", h=H),
                        axis=mybir.AxisListType.X,
                    )
                    nrm = stats.tile([128, H], F32, tag="nrm")
                    nc.scalar.activation(
                        nrm[:], ssq[:], mybir.ActivationFunctionType.Sqrt,
                        bias=0.0, scale=1.0,
                    )
                    rinv = stats.tile([128, H], F32, tag="rinv")
                    nc.vector.reciprocal(rinv[:], nrm[:])

                    q3 = qtmp[:].rearrange("p (h d) -> p h d", h=H)
                    t1 = pb.tile([128, INNER], F16, tag="t1", bufs=2)
                    nc.vector.tensor_mul(
                        t1[:].rearrange("p (h d) -> p h d", h=H),
                        q3, _bcast_heads(cos_sb[:, m, :]),
                    )
                    t2 = pb.tile([128, INNER], F16, tag="t2", bufs=2)
                    sin_b = bass.AP(
                        tensor=sin_sb.tensor,
                        offset=sin_sb[:, m, :].offset,
                        ap=[sin_sb[:, m, :].ap[0], [0, H], [32, 2], [16, 2],
                            [1, 16]],
                    )
                    nc.vector.tensor_mul(
                        t2[:].rearrange("p (h a b c) -> p h a b c",
                                        h=H, a=2, b=2, c=16),
                        _rot_view(qtmp[:]), sin_b,
                    )
                    if dve_add:
                        # split the rope add across DVE and Pool so the Pool
                        # engine (2x slower per element) stops pacing the loop
                        nc.vector.tensor_add(t1[:, 0:512], t1[:, 0:512], t2[:, 0:512])
                        nc.gpsimd.tensor_add(t1[:, 512:1024], t1[:, 512:1024], t2[:, 512:1024])
                    else:
                        nc.gpsimd.tensor_add(t1[:], t1[:], t2[:])
                    qr = pb.tile([128, INNER], F16, tag="qr", bufs=4)
                    nc.gpsimd.tensor_mul(
                        qr[:].rearrange("p (h d) -> p h d", h=H),
                        t1[:].rearrange("p (h d) -> p h d", h=H),
                        _bcast_last(rinv[:], DH),
                    )
                    return qr

                def transp(qr, m, dst):
                    for g in range(2):
                        tps = tp.tile([128, 512], F16, tag="tp", bufs=4)
                        for b4 in range(4):
                            f = g * 4 + b4
                            nc.tensor.transpose(
                                tps[:, b4 * 128:(b4 + 1) * 128],
                                qr[:, f * 128:(f + 1) * 128],
                                ident_h[:],
                            )
                        nc.scalar.copy(
                            dst[:, g * 4:(g + 1) * 4, m * 128:(m + 1) * 128],
                            tps[:].rearrange("p (a t) -> p a t", a=4),
                        )

                # ---- Q loop (LayerNorm fused; LN math runs two iterations
                # ahead and RoPE-output transposes two behind so the PE
                # in-order queue never waits on the ACT/DVE/GPS chains) ----
                xns = {0: ln_math(0), 1: ln_math(1)}
                # rope outputs pending PE transposition; the 3-deep queue
                # carries across the Q->K boundary so the K projections fill
                # the wait on Q's last rope chains instead of the PE idling
                pending = []
                w_k = None
                # transpose m+1's xn tile while proj(m) runs so proj never
                # heads-of-line blocks on its own xnT eviction
                ln_transp(xns.pop(0), 0)
                for m in range(MT):
                    if m + 1 < MT:
                        ln_transp(xns.pop(m + 1), m + 1)
                    ps = proj(w_q, m, pp, bq_sb, 0)
                    if m == 0:
                        w_k = stream_w(wk_d)   # lands during the q loop
                    if len(pending) >= 3:
                        transp(*pending.pop(0))
                    pending.append((rms_rope(ps, m), m, qT))
                    if m + 2 < MT:
                        xns[m + 2] = ln_math(m + 2)

                # ---- K loop ----
                w_v = None
                for m in range(MT):
                    ps = proj(w_k, m, pp, bkv_sb, 0)
                    if m == 0:
                        w_v = stream_w(wv_d)
                    if len(pending) >= 3:
                        transp(*pending.pop(0))
                    pending.append((rms_rope(ps, m, dve_add=True), m, kT))
                es_store = {}

                def qk_exp_h(h, j, dpool):
                    """QK + exp for head h, key tile j. One [128, N] dots
                    tile (2 banks) per step, double-buffered: QK(s+1) runs
                    during exp(s), so exp never waits on the PE."""
                    f, pb_ = h >> 1, (h & 1) * 64
                    dots = dpool.tile([128, N], F32, tag="dots", bufs=2)
                    for qh in range(2):
                        qsl = slice(qh * 512, (qh + 1) * 512)
                        nc.tensor.matmul(
                            dots[:, qsl],
                            kT[pb_:pb_ + 64, f, j * 128:(j + 1) * 128],
                            qT[pb_:pb_ + 64, f, qsl],
                            start=True, stop=True,
                        )
                    e_t = ep.tile([128, N], BF16, tag="E", bufs=24)
                    nc.scalar.activation(
                        e_t[:], dots[:], mybir.ActivationFunctionType.Exp)
                    es_store[(h, j)] = e_t

                outT = xnT  # xnT dead after projections; reuse for attn out

                def av_alloc_h(pool, tag, bufs=4):
                    return [pool.tile([128, 512], F32, tag=tag, bufs=bufs,
                                      name=f"oa{qh}") for qh in range(2)]

                def av_step_h(h, j, oas2):
                    for qh in range(2):
                        nc.tensor.matmul(
                            oas2[qh], v_sb[:, j, h, :],
                            es_store[(h, j)][:, qh * 512:(qh + 1) * 512],
                            start=(j == 0), stop=(j == MT - 1),
                        )

                def av_finish_h(h, oas2):
                    """Softmax-normalize: psum rows 64-127 hold 64 identical
                    copies of the denominator (ones half of the V stationary);
                    recip them on DVE and fuse the divide into the psum
                    eviction, writing fp16 straight into outT."""
                    f, pb_ = h >> 1, (h & 1) * 64
                    scs = []
                    for qh in range(2):
                        sc = cpool.tile([128, 512], BF16, tag="sc", bufs=4)
                        nc.vector.tensor_copy(sc[:], oas2[qh][:])
                        scs.append(sc)
                    for qh, sc in enumerate(scs):
                        dn = cpool.tile([64, 512], F32, tag="dn", bufs=2)
                        nc.vector.tensor_copy(dn[:], sc[64:128, :])
                        rq = cpool.tile([64, 512], F32, tag="rq", bufs=2)
                        nc.vector.reciprocal_approx_fast(rq[:], dn[:])
                        nc.vector.tensor_mul(
                            outT[pb_:pb_ + 64, f, qh * 512:(qh + 1) * 512],
                            sc[0:DH, :], rq[:],
                        )

                oas_map = {}

                def av_sched(h_av, j, pool, tag, bufs=4):
                    if j == 0:
                        oas_map[h_av] = av_alloc_h(pool, tag, bufs)
                    av_step_h(h_av, j, oas_map[h_av])
                    if j == MT - 1:
                        av_finish_h(h_av, oas_map.pop(h_av))

                def v_proj(m):
                    for nh in range(2):
                        ps = vpool.tile([128, 512], F32, tag="vp", bufs=2,
                                        name=f"vp{nh}")
                        sl = slice(nh * 512, (nh + 1) * 512)
                        if bkv_sb is not None:
                            nc.tensor.matmul(
                                ps[:], ones1[:],
                                bkv_sb[:, INNER + nh * 512:
                                       INNER + (nh + 1) * 512],
                                start=True, stop=False,
                            )
                        for k in range(KT):
                            nc.tensor.matmul(
                                ps[:], xnT[:, k, m * 128:(m + 1) * 128],
                                w_v[:, k, sl],
                                start=(k == 0 and bkv_sb is None),
                                stop=(k == KT - 1),
                            )
                        nc.vector.tensor_copy(
                            v_sb[:, m, nh * 8:(nh + 1) * 8, 0:DH],
                            ps[:].rearrange("p (h d) -> p h d", h=8),
                        )

                # ---- k tail + V loop with heads 0-2 QK/exp and av(0) ----
                # All three k-tail transposes drain, then the A/B psum pools
                # close so the V projections (1-bank halves), pair-0 dots and
                # av(0)'s accumulators fit in 8 banks together.
                transp(*pending.pop(0))
                transp(*pending.pop(0))
                transp(*pending.pop(0))
                tp_cm.__exit__(None, None, None)
                pp_cm.__exit__(None, None, None)
                vpool_cm = tc.tile_pool(name="vpool", bufs=2, space="PSUM")
                vpool = vpool_cm.__enter__()
                opv_cm = tc.tile_pool(name="opv", bufs=2, space="PSUM")
                opv = opv_cm.__enter__()
                dp0_cm = tc.tile_pool(name="dp0", bufs=2, space="PSUM")
                dp0 = dp0_cm.__enter__()
                v_proj(0)

                w_o = None
                # 3 QK/exp steps per v_proj block so ACT (~3.3us of exp)
                # stays fed through each 3.4us projection; av(0) rides the
                # back half once its es tiles and v_sb key-tiles exist.
                sched = {1: [('q', 0, 0), ('q', 0, 1), ('q', 0, 2)],
                         2: [('q', 0, 3), ('q', 0, 4), ('q', 0, 5)],
                         3: [('q', 0, 6), ('q', 0, 7), ('q', 1, 0)],
                         4: [('q', 1, 1), ('q', 1, 2), ('q', 1, 3)],
                         5: [('q', 1, 4), ('q', 1, 5), ('q', 1, 6),
                             ('a', 0, 0)],
                         6: [('q', 1, 7), ('q', 2, 0), ('q', 2, 1),
                             ('a', 0, 1), ('a', 0, 2)],
                         7: [('q', 2, 2), ('q', 2, 3), ('q', 2, 4),
                             ('a', 0, 3), ('a', 0, 4)]}
                for m in range(1, MT):
                    v_proj(m)
                    if m == 1:
                        w_o = stream_w(wo_d)
                    for kind, hh, j in sched.get(m, []):
                        if kind == 'q':
                            qk_exp_h(hh, j, dp0)
                        else:
                            av_sched(hh, j, opv, "oav", bufs=2)
                # boundary fill: av(0) tail + two more head-2 steps
                av_sched(0, 5, opv, "oav", bufs=2)
                qk_exp_h(2, 5, dp0)
                av_sched(0, 6, opv, "oav", bufs=2)
                qk_exp_h(2, 6, dp0)
                av_sched(0, 7, opv, "oav", bufs=2)
                dp0_cm.__exit__(None, None, None)
                opv_cm.__exit__(None, None, None)
                vpool_cm.__exit__(None, None, None)

            # ---------------- Phase C: attention ----------------
            if True:
                op_cm = tc.tile_pool(name="op", bufs=4, space="PSUM")
                op = op_cm.__enter__()
                dp_cm = tc.tile_pool(name="dp", bufs=2, space="PSUM")
                dp = dp_cm.__enter__()

                # AV for head h-1 rides along with QK/exp of head h (av(0)
                # already ran inside the V loop); the oa tag's 4 slots hold
                # two heads' accumulators (one accumulating, one finishing).
                for h in range(2, H):
                    for j in range(MT):
                        av_sched(h - 1, j, op, "oa")
                        if (h, j) not in es_store:
                            qk_exp_h(h, j, dp)
                # dots banks are dead after the last QK: close dp and give
                # the last head's AV fresh psum slots so the tail starts
                # without waiting on earlier eviction chains.
                dp_cm.__exit__(None, None, None)
                opx_cm = tc.tile_pool(name="opx", bufs=4, space="PSUM")
                opx = opx_cm.__enter__()
                for j in range(MT):
                    av_sched(H - 1, j, opx, "oax")
                opx_cm.__exit__(None, None, None)
                op_cm.__exit__(None, None, None)

            # ---------------- Phase D: Wo projection ----------------
            with (
                tc.tile_pool(name="fin", bufs=2) as fin,
                tc.tile_pool(name="fp", bufs=3, space="PSUM") as fp,
            ):
                for m in range(MT):
                    ps = fp.tile([128, DIM], F32, tag="fp", bufs=3)
                    for f in range(FT):
                        for nh in range(2):
                            sl = slice(nh * 512, (nh + 1) * 512)
                            nc.tensor.matmul(
                                ps[:, sl],
                                outT[:, f, m * 128:(m + 1) * 128],
                                w_o[:, f, nh * 512:(nh + 1) * 512],
                                start=(f == 0), stop=(f == FT - 1),
                            )
                    fs = fin.tile([128, DIM], F32, tag="fs", bufs=2)
                    for nh in range(2):
                        sl = slice(nh * 512, (nh + 1) * 512)
                        if nh == 0:
                            nc.scalar.copy(fs[:, sl], ps[:, sl])
                        else:
                            nc.vector.tensor_copy(fs[:, sl], ps[:, sl])
                    nc.sync.dma_start(out_d[m * 128:(m + 1) * 128, :], fs[:])

            cp_cm.__exit__(None, None, None)
            ep_cm.__exit__(None, None, None)

    nc.compile()
    return nc


import numpy as np
from concourse.bass_utils import run_bass_kernel_spmd

_NC_CACHE = {}


def _get_nc(has_bias: bool):
    if has_bias not in _NC_CACHE:
        _NC_CACHE[has_bias] = build_nc(has_bias)
    return _NC_CACHE[has_bias]


def _pmajor(w):
    """[DIM, C] -> [128, KT*C] partition-major relayout (contiguous DMA)."""
    d, c = w.shape
    kt = d // 128
    return np.ascontiguousarray(
        w.reshape(kt, 128, c).transpose(1, 0, 2).reshape(128, kt * c))


def host_prepare(x, mask, h_idx, w_idx, gamma_ln, beta_ln, q_gamma, k_gamma,
                 Wq, Wkv, Wo):
    x = np.asarray(x, np.float32)
    mask = np.asarray(mask)
    assert mask.all(), "kernel assumes all-True mask"
    assert np.allclose(np.asarray(q_gamma), 1.0), "kernel assumes q_gamma == 1"
    assert np.allclose(np.asarray(k_gamma), 1.0), "kernel assumes k_gamma == 1"

    gamma_ln = np.asarray(gamma_ln, np.float32)
    beta_ln = np.asarray(beta_ln, np.float32)
    Wq = np.asarray(Wq, np.float32)
    Wkv = np.asarray(Wkv, np.float32)
    Wo = np.asarray(Wo, np.float32)

    wq_f = _pmajor((gamma_ln[:, None] * Wq).astype(np.float16))
    wkv_g = (gamma_ln[:, None] * Wkv).astype(np.float16)
    wk_f = _pmajor(wkv_g[:, :INNER])
    wv_f = _pmajor(wkv_g[:, INNER:])
    wo_f = _pmajor(Wo.astype(np.float16))
    bq = (beta_ln @ Wq)[None, :].astype(np.float32)
    bkv = (beta_ln @ Wkv)[None, :].astype(np.float32)
    has_bias = bool(np.abs(bq).max() > 0 or np.abs(bkv).max() > 0)

    # RoPE tables; sqrt(DH)=8 and rotate-half signs folded in.
    h_idx = np.asarray(h_idx, np.float32)
    w_idx = np.asarray(w_idx, np.float32)
    dq = DH // 4
    inv_freq = 1.0 / (10000.0 ** (np.arange(dq, dtype=np.float32) / dq))
    th = h_idx[..., None] * inv_freq
    tw = w_idx[..., None] * inv_freq
    cos_t = (np.concatenate([np.cos(th), np.cos(th), np.cos(tw), np.cos(tw)], -1)
             * np.sqrt(np.float32(DH))).astype(np.float16)
    sin_full = (np.concatenate([np.sin(th), np.sin(th), np.sin(tw), np.sin(tw)], -1)
                * np.sqrt(np.float32(DH)))
    sign = np.tile(np.concatenate(
        [-np.ones(dq, np.float32), np.ones(dq, np.float32)]), 2)
    sin_t = (sin_full * sign).astype(np.float16)

    def trig_pmajor(t):
        # [N, DH] -> [128, MT*DH] partition-major
        return np.ascontiguousarray(
            t.reshape(MT, 128, DH).transpose(1, 0, 2).reshape(128, MT * DH))

    in_maps = []
    for b in range(B):
        m = {
            "x": np.ascontiguousarray(x[b]).astype(np.float16),
            "ident16": np.eye(128, dtype=np.float16),
            "wq": wq_f,
            "wk": wk_f,
            "wv": wv_f,
            "wo": wo_f,
            "cos_t": trig_pmajor(cos_t[b]),
            "sin_t": trig_pmajor(sin_t[b]),
        }
        if has_bias:
            m["bq"] = bq.astype(np.float16)
            m["bkv"] = bkv.astype(np.float16)
        in_maps.append(m)
    return in_maps, has_bias


def run(trace=False, **inputs):
    in_maps, has_bias = host_prepare(**inputs)
    nc = _get_nc(has_bias)
    res = run_bass_kernel_spmd(nc, in_maps, core_ids=list(range(B)), trace=trace)
    out = np.stack([res.results[c]["out"] for c in range(B)], axis=0)
    return out.astype(np.float32), res


def kernel(**inputs):
    out, _ = run(trace=False, **inputs)
    return out


if __name__ == "__main__":
    build_nc(False)
    print("build ok")



# revision 81
# speedup vs baseline: 1.0620x; 1.0620x over previous
"""Trainium2 Bass kernel for nn_Attention_22299470201527.

Dense transformer attention block:
  LayerNorm -> Wq/Wkv projections -> per-head QK RMSNorm -> 2D RoPE ->
  softmax(QK^T) V -> Wo projection,  B=8, N=1024, DIM=1024, H=16, DH=64.

Sharding: data-parallel over batch — 8 batch elements on 8 NeuronCores,
one per core, weights replicated, no collectives. kernel(**inputs) takes
the full unsharded inputs and returns the full [8, 1024, 1024] output.

Design notes (~313us HW exec; pair-granular predecessor 341us, f32r
baseline 397us):
  * All matmul operands fp16/bf16 (FWL weight loads, half the DMA bytes).
  * Weights/trig tables are relaid out host-side to partition-major so
    every DMA lands as 128 large contiguous descriptors.
  * x tiles are prefetched ahead of the weight streams; each weight
    matrix streams during the previous projection loop (wpool bufs=2).
  * LayerNorm is fused into the Q loop; LN math runs two iterations
    ahead, xn transposes one ahead of proj, and the RoPE-output PE
    transposes three behind, so the in-order PE queue rarely blocks on
    the eviction -> square -> reduce -> sqrt -> RoPE -> rinv chain.
  * Attention runs at HEAD granularity: one [128 keys, 1024 q] dots
    psum tile (2 banks) per (head, key-tile) step, double-buffered, so
    QK of step s+1 overlaps exp of step s and the ACT engine streams
    exp back-to-back at ~1.05us/tile — ACT is the phase-C roofline
    (128 exps ~ 138us).  PSUM: dots 2x2 banks + AV accumulators 4x1.
  * AV for head h-1 rides along with QK/exp of head h; the last head's
    AV tail runs on fresh psum banks (dots pool closed) to avoid WAR
    waits and a p-state dip.
  * Softmax denominators: the V stationary is [dh(64) | ones(64)], so
    AV psum rows 64-127 come out as 64 pre-broadcast copies of the
    denominator.  Normalization = one bf16 psum eviction (frees the
    bank fast) + reciprocal_approx_fast + multiply, all on DVE, with
    fp16 results written into the dead xnT buffer for Wo.  (No DRAM
    bounce, no partition broadcast: approx_fast only reads base-0
    SBUF — it silently returns garbage on PSUM or partition-64 APs.)
  * Heads 0-2 pre-compute QK/exp inside the V-projection loop (3 exp
    steps per 3.4us projection block keeps ACT fed), with av(0) riding
    the back half on its own psum bank pair (V projections run as
    1-bank halves to make room), so phase C starts with zero backlog.
  * Wo accumulates per token tile with a 3-deep psum pipeline; output
    eviction splits ACT/DVE halves, one 512KB DMA per tile.
"""

import sys

for _p in ("/opt/trn_rl_repo",):
    if _p not in sys.path:
        sys.path.append(_p)

import concourse.bacc as bacc
import concourse.bass as bass
import concourse.tile as tile
from concourse import mybir

F32 = mybir.dt.float32
F16 = mybir.dt.float16
BF16 = mybir.dt.bfloat16

B, N, DIM, H, DH = 8, 1024, 1024, 16, 64
INNER = H * DH
KT = DIM // 128
MT = N // 128
FT = INNER // 128
EPS_LN = 1e-5
EPS_NORM = 1e-12


def _bcast_heads(ap2d, nheads=H):
    """[128, D] AP -> [128, nheads, D], stride-0 broadcast over heads."""
    return bass.AP(
        tensor=ap2d.tensor, offset=ap2d.offset,
        ap=[ap2d.ap[0], [0, nheads], ap2d.ap[1]],
    )


def _bcast_last(ap2d, n):
    """[128, Hn] AP -> [128, Hn, n], stride-0 broadcast innermost."""
    return bass.AP(
        tensor=ap2d.tensor, offset=ap2d.offset,
        ap=[ap2d.ap[0], ap2d.ap[1], [0, n]],
    )


def _rot_view(tile_ap):
    """[128, 1024] tile viewed [128, H, 2, 2, 16] with adjacent 16-blocks
    swapped (rotate-half shuffle; signs live in the sin table)."""
    return bass.AP(
        tensor=tile_ap.tensor, offset=tile_ap.offset + 16,
        ap=[tile_ap.ap[0], [DH, H], [32, 2], [-16, 2], [1, 16]],
    )


def build_nc(has_bias: bool):
    nc = bacc.Bacc("TRN2", target_bir_lowering=False, debug=False, num_devices=8)

    x_d = nc.dram_tensor("x", [N, DIM], F16, kind="ExternalInput")
    # weights partition-major: [128, KT, INNER] flattened per partition
    wq_d = nc.dram_tensor("wq", [128, KT * INNER], F16, kind="ExternalInput")
    wk_d = nc.dram_tensor("wk", [128, KT * INNER], F16, kind="ExternalInput")
    wv_d = nc.dram_tensor("wv", [128, KT * INNER], F16, kind="ExternalInput")
    wo_d = nc.dram_tensor("wo", [128, KT * INNER], F16, kind="ExternalInput")
    id16_d = nc.dram_tensor("ident16", [128, 128], F16, kind="ExternalInput")
    cos_d = nc.dram_tensor("cos_t", [128, MT * DH], F16, kind="ExternalInput")
    sin_d = nc.dram_tensor("sin_t", [128, MT * DH], F16, kind="ExternalInput")
    if has_bias:
        bq_d = nc.dram_tensor("bq", [1, INNER], F16, kind="ExternalInput")
        bkv_d = nc.dram_tensor("bkv", [1, 2 * INNER], F16, kind="ExternalInput")
    out_d = nc.dram_tensor("out", [N, DIM], F32, kind="ExternalOutput")

    with tile.TileContext(nc) as tc:
        with (
            tc.tile_pool(name="const", bufs=1) as constp,
            tc.tile_pool(name="wpool", bufs=2) as wpool,
            tc.tile_pool(name="stats", bufs=2) as stats,
            tc.tile_pool(name="bc", bufs=1) as bc,
            tc.tile_pool(name="xa", bufs=1) as xa,
        ):
            # Prefetch first x tiles BEFORE everything else so LayerNorm
            # starts immediately.
            XPRE = 2
            x_tiles = {}

            def x_fetch(m):
                x_t = xa.tile([128, DIM], F16, tag="x", bufs=2, name=f"x{m%2}")
                nc.sync.dma_start(x_t[:], x_d[m * 128:(m + 1) * 128, :])
                x_tiles[m] = x_t

            for m in range(XPRE):
                x_fetch(m)

            ident_h = constp.tile([128, 128], F16)
            nc.sync.dma_start(ident_h[:], id16_d[:])
            eps_t = constp.tile([128, 1], F32)
            nc.vector.memset(eps_t[:], EPS_LN)

            def stream_w(dram_t):
                w = wpool.tile([128, KT, INNER], F16, tag="w")
                nc.sync.dma_start(
                    w[:], dram_t[:].rearrange("p (a i) -> p a i", a=KT)
                )
                return w

            w_q = stream_w(wq_d)

            cos_sb = constp.tile([128, MT, DH], F16)
            sin_sb = constp.tile([128, MT, DH], F16)
            nc.sync.dma_start(cos_sb[:], cos_d[:].rearrange("p (a d) -> p a d", a=MT))
            nc.sync.dma_start(sin_sb[:], sin_d[:].rearrange("p (a d) -> p a d", a=MT))
            bq_sb = bkv_sb = ones1 = None
            if has_bias:
                bq_sb = constp.tile([1, INNER], F16)
                bkv_sb = constp.tile([1, 2 * INNER], F16)
                nc.sync.dma_start(bq_sb[:], bq_d[:])
                nc.sync.dma_start(bkv_sb[:], bkv_d[:])
                ones1 = constp.tile([1, 128], F16)
                nc.vector.memset(ones1[:], 1.0)

            # Long-lived activations.
            qT = bc.tile([128, FT, N], F16)
            kT = bc.tile([128, FT, N], F16)
            xnT = bc.tile([128, KT, N], F16)   # reused as outT in phase C/D
            # V stationary is [dh (64) | ones (64)] per head: AV psum rows
            # 64-127 come out as 64 pre-broadcast copies of the softmax
            # denominator, so normalization needs no partition broadcast.
            v_sb = bc.tile([128, MT, H, 2 * DH], BF16)
            nc.gpsimd.memset(
                bass.AP(
                    tensor=v_sb.tensor, offset=v_sb[:].offset + DH,
                    ap=[v_sb[:].ap[0], [H * 2 * DH, MT], [2 * DH, H],
                        [1, DH]],
                ),
                1.0,
            )

            ep_cm = tc.tile_pool(name="ep", bufs=1)
            ep = ep_cm.__enter__()
            cp_cm = tc.tile_pool(name="cpool", bufs=1)
            cpool = cp_cm.__enter__()

            with tc.tile_pool(name="pb", bufs=1) as pb:
                pp_cm = tc.tile_pool(name="pp", bufs=2, space="PSUM")
                pp = pp_cm.__enter__()
                tp_cm = tc.tile_pool(name="tp", bufs=4, space="PSUM")
                tp = tp_cm.__enter__()
                def ln_math(m):
                    """LayerNorm stats + normalized fp16 tile for x tile m."""
                    x_t = x_tiles.pop(m)
                    st = stats.tile([128, 2, 6], F32, tag="bst")
                    for g in range(2):
                        nc.vector.bn_stats(st[:, g, :], x_t[:, g * 512:(g + 1) * 512])
                    mv = stats.tile([128, 2], F32, tag="mv")
                    nc.vector.bn_aggr(mv[:], st[:])
                    sd = stats.tile([128, 1], F32, tag="sd")
                    nc.scalar.activation(
                        sd[:], mv[:, 1:2], mybir.ActivationFunctionType.Sqrt,
                        bias=eps_t[:], scale=1.0,
                    )
                    rstd = stats.tile([128, 1], F32, tag="rstd")
                    nc.vector.reciprocal(rstd[:], sd[:])
                    nmu = stats.tile([128, 1], F32, tag="nmu")
                    nc.vector.scalar_tensor_tensor(
                        out=nmu[:], in0=mv[:, 0:1], scalar=-1.0, in1=rstd[:],
                        op0=mybir.AluOpType.mult, op1=mybir.AluOpType.mult,
                    )
                    xn_t = xa.tile([128, DIM], F16, tag="xn", bufs=3)
                    nc.scalar.activation(
                        xn_t[:], x_t[:], mybir.ActivationFunctionType.Identity,
                        bias=nmu[:], scale=rstd[:],
                    )
                    if m + XPRE < MT:
                        x_fetch(m + XPRE)
                    return xn_t

                def ln_transp(xn_t, m):
                    for g in range(2):
                        tps = tp.tile([128, 512], F16, tag="tp", bufs=4)
                        for b4 in range(4):
                            k = g * 4 + b4
                            nc.tensor.transpose(
                                tps[:, b4 * 128:(b4 + 1) * 128],
                                xn_t[:, k * 128:(k + 1) * 128],
                                ident_h[:],
                            )
                        # split the two evictions across ACT and DVE so the
                        # dependent proj(m) isn't gated by the ACT queue
                        dst = xnT[:, g * 4:(g + 1) * 4, m * 128:(m + 1) * 128]
                        src = tps[:].rearrange("p (a t) -> p a t", a=4)
                        if g == 0:
                            nc.scalar.copy(dst, src)
                        else:
                            nc.vector.tensor_copy(dst, src)

                def proj(w, m, psp, bias_sb=None, bias_off=0):
                    """One [128, INNER] projection psum tile for token tile m.
                    Matmul moving free dim is capped at 512, so each half is
                    its own accumulation group."""
                    ps = psp.tile([128, INNER], F32, tag="pp", bufs=2)
                    for nh in range(2):
                        sl = slice(nh * 512, (nh + 1) * 512)
                        if bias_sb is not None:
                            nc.tensor.matmul(
                                ps[:, sl], ones1[:],
                                bias_sb[:, bias_off + nh * 512:
                                        bias_off + (nh + 1) * 512],
                                start=True, stop=False,
                            )
                        for k in range(KT):
                            nc.tensor.matmul(
                                ps[:, sl],
                                xnT[:, k, m * 128:(m + 1) * 128],
                                w[:, k, nh * 512:(nh + 1) * 512],
                                start=(k == 0 and bias_sb is None),
                                stop=(k == KT - 1),
                            )
                    return ps

                def rms_rope(ps, m, dve_add=False):
                    qtmp = pb.tile([128, INNER], F16, tag="qtmp", bufs=2)
                    nc.scalar.copy(qtmp[:], ps[:])
                    sq = pb.tile([128, INNER], F16, tag="sq", bufs=1)
                    nc.scalar.activation(
                        sq[:], qtmp[:], mybir.ActivationFunctionType.Square,
                        bias=0.0, scale=1.0,
                    )
                    ssq = stats.tile([128, H], F32, tag="ssq")
                    nc.vector.reduce_sum(
                        ssq[:], sq[:].rearrange("p (h d) -> p h d", h=H),
                        axis=mybir.AxisListType.X,
                    )
                    nrm = stats.tile([128, H], F32, tag="nrm")
                    nc.scalar.activation(
                        nrm[:], ssq[:], mybir.ActivationFunctionType.Sqrt,
                        bias=0.0, scale=1.0,
                    )
                    rinv = stats.tile([128, H], F32, tag="rinv")
                    nc.vector.reciprocal(rinv[:], nrm[:])

                    q3 = qtmp[:].rearrange("p (h d) -> p h d", h=H)
                    t1 = pb.tile([128, INNER], F16, tag="t1", bufs=2)
                    nc.vector.tensor_mul(
                        t1[:].rearrange("p (h d) -> p h d", h=H),
                        q3, _bcast_heads(cos_sb[:, m, :]),
                    )
                    t2 = pb.tile([128, INNER], F16, tag="t2", bufs=2)
                    sin_b = bass.AP(
                        tensor=sin_sb.tensor,
                        offset=sin_sb[:, m, :].offset,
                        ap=[sin_sb[:, m, :].ap[0], [0, H], [32, 2], [16, 2],
                            [1, 16]],
                    )
                    nc.vector.tensor_mul(
                        t2[:].rearrange("p (h a b c) -> p h a b c",
                                        h=H, a=2, b=2, c=16),
                        _rot_view(qtmp[:]), sin_b,
                    )
                    if dve_add:
                        # split the rope add across DVE and Pool so the Pool
                        # engine (2x slower per element) stops pacing the loop
                        nc.vector.tensor_add(t1[:, 0:512], t1[:, 0:512], t2[:, 0:512])
                        nc.gpsimd.tensor_add(t1[:, 512:1024], t1[:, 512:1024], t2[:, 512:1024])
                    else:
                        nc.gpsimd.tensor_add(t1[:], t1[:], t2[:])
                    qr = pb.tile([128, INNER], F16, tag="qr", bufs=4)
                    nc.gpsimd.tensor_mul(
                        qr[:].rearrange("p (h d) -> p h d", h=H),
                        t1[:].rearrange("p (h d) -> p h d", h=H),
                        _bcast_last(rinv[:], DH),
                    )
                    return qr

                def transp(qr, m, dst):
                    for g in range(2):
                        tps = tp.tile([128, 512], F16, tag="tp", bufs=4)
                        for b4 in range(4):
                            f = g * 4 + b4
                            nc.tensor.transpose(
                                tps[:, b4 * 128:(b4 + 1) * 128],
                                qr[:, f * 128:(f + 1) * 128],
                                ident_h[:],
                            )
                        nc.scalar.copy(
                            dst[:, g * 4:(g + 1) * 4, m * 128:(m + 1) * 128],
                            tps[:].rearrange("p (a t) -> p a t", a=4),
                        )

                # ---- Q loop (LayerNorm fused; LN math runs two iterations
                # ahead and RoPE-output transposes two behind so the PE
                # in-order queue never waits on the ACT/DVE/GPS chains) ----
                xns = {0: ln_math(0), 1: ln_math(1)}
                # rope outputs pending PE transposition; the 3-deep queue
                # carries across the Q->K boundary so the K projections fill
                # the wait on Q's last rope chains instead of the PE idling
                pending = []
                w_k = None
                # transpose m+1's xn tile while proj(m) runs so proj never
                # heads-of-line blocks on its own xnT eviction
                ln_transp(xns.pop(0), 0)
                for m in range(MT):
                    if m + 1 < MT:
                        ln_transp(xns.pop(m + 1), m + 1)
                    ps = proj(w_q, m, pp, bq_sb, 0)
                    if m == 0:
                        w_k = stream_w(wk_d)   # lands during the q loop
                    if len(pending) >= 3:
                        transp(*pending.pop(0))
                    pending.append((rms_rope(ps, m), m, qT))
                    if m + 2 < MT:
                        xns[m + 2] = ln_math(m + 2)

                # ---- K loop ----
                w_v = None
                for m in range(MT):
                    ps = proj(w_k, m, pp, bkv_sb, 0)
                    if m == 0:
                        w_v = stream_w(wv_d)
                    if len(pending) >= 3:
                        transp(*pending.pop(0))
                    pending.append((rms_rope(ps, m, dve_add=True), m, kT))
                es_store = {}

                def qk_exp_h(h, j, dpool):
                    """QK + exp for head h, key tile j. One [128, N] dots
                    tile (2 banks) per step, double-buffered: QK(s+1) runs
                    during exp(s), so exp never waits on the PE."""
                    f, pb_ = h >> 1, (h & 1) * 64
                    dots = dpool.tile([128, N], F32, tag="dots", bufs=2)
                    for qh in range(2):
                        qsl = slice(qh * 512, (qh + 1) * 512)
                        nc.tensor.matmul(
                            dots[:, qsl],
                            kT[pb_:pb_ + 64, f, j * 128:(j + 1) * 128],
                            qT[pb_:pb_ + 64, f, qsl],
                            start=True, stop=True,
                        )
                    e_t = ep.tile([128, N], BF16, tag="E", bufs=24)
                    nc.scalar.activation(
                        e_t[:], dots[:], mybir.ActivationFunctionType.Exp)
                    es_store[(h, j)] = e_t

                outT = xnT  # xnT dead after projections; reuse for attn out

                def av_alloc_h(pool, tag, bufs=4):
                    return [pool.tile([128, 512], F32, tag=tag, bufs=bufs,
                                      name=f"oa{qh}") for qh in range(2)]

                def av_step_h(h, j, oas2):
                    for qh in range(2):
                        nc.tensor.matmul(
                            oas2[qh], v_sb[:, j, h, :],
                            es_store[(h, j)][:, qh * 512:(qh + 1) * 512],
                            start=(j == 0), stop=(j == MT - 1),
                        )

                def av_finish_h(h, oas2):
                    """Softmax-normalize: psum rows 64-127 hold 64 identical
                    copies of the denominator (ones half of the V stationary);
                    recip them on DVE and fuse the divide into the psum
                    eviction, writing fp16 straight into outT."""
                    f, pb_ = h >> 1, (h & 1) * 64
                    scs = []
                    for qh in range(2):
                        sc = cpool.tile([128, 512], BF16, tag="sc", bufs=4)
                        nc.vector.tensor_copy(sc[:], oas2[qh][:])
                        scs.append(sc)
                    for qh, sc in enumerate(scs):
                        dn = cpool.tile([64, 512], F32, tag="dn", bufs=2)
                        nc.vector.tensor_copy(dn[:], sc[64:128, :])
                        rq = cpool.tile([64, 512], F32, tag="rq", bufs=2)
                        nc.vector.reciprocal_approx_fast(rq[:], dn[:])
                        nc.vector.tensor_mul(
                            outT[pb_:pb_ + 64, f, qh * 512:(qh + 1) * 512],
                            sc[0:DH, :], rq[:],
                        )

                oas_map = {}

                def av_sched(h_av, j, pool, tag, bufs=4):
                    if j == 0:
                        oas_map[h_av] = av_alloc_h(pool, tag, bufs)
                    av_step_h(h_av, j, oas_map[h_av])
                    if j == MT - 1:
                        av_finish_h(h_av, oas_map.pop(h_av))

                def v_proj(m):
                    for nh in range(2):
                        ps = vpool.tile([128, 512], F32, tag="vp", bufs=2,
                                        name=f"vp{nh}")
                        sl = slice(nh * 512, (nh + 1) * 512)
                        if bkv_sb is not None:
                            nc.tensor.matmul(
                                ps[:], ones1[:],
                                bkv_sb[:, INNER + nh * 512:
                                       INNER + (nh + 1) * 512],
                                start=True, stop=False,
                            )
                        for k in range(KT):
                            nc.tensor.matmul(
                                ps[:], xnT[:, k, m * 128:(m + 1) * 128],
                                w_v[:, k, sl],
                                start=(k == 0 and bkv_sb is None),
                                stop=(k == KT - 1),
                            )
                        nc.vector.tensor_copy(
                            v_sb[:, m, nh * 8:(nh + 1) * 8, 0:DH],
                            ps[:].rearrange("p (h d) -> p h d", h=8),
                        )

                # ---- k tail + V loop with heads 0-2 QK/exp and av(0) ----
                # All three k-tail transposes drain, then the A/B psum pools
                # close so the V projections (1-bank halves), pair-0 dots and
                # av(0)'s accumulators fit in 8 banks together.
                transp(*pending.pop(0))
                transp(*pending.pop(0))
                transp(*pending.pop(0))
                tp_cm.__exit__(None, None, None)
                pp_cm.__exit__(None, None, None)
                vpool_cm = tc.tile_pool(name="vpool", bufs=2, space="PSUM")
                vpool = vpool_cm.__enter__()
                opv_cm = tc.tile_pool(name="opv", bufs=2, space="PSUM")
                opv = opv_cm.__enter__()
                dp0_cm = tc.tile_pool(name="dp0", bufs=2, space="PSUM")
                dp0 = dp0_cm.__enter__()
                v_proj(0)

                w_o = None
                # 3 QK/exp steps per v_proj block so ACT (~3.3us of exp)
                # stays fed through each 3.4us projection; av(0) rides the
                # back half once its es tiles and v_sb key-tiles exist.
                sched = {1: [('q', 0, 0), ('q', 0, 1), ('q', 0, 2)],
                         2: [('q', 0, 3), ('q', 0, 4), ('q', 0, 5)],
                         3: [('q', 0, 6), ('q', 0, 7), ('q', 1, 0)],
                         4: [('q', 1, 1), ('q', 1, 2), ('q', 1, 3)],
                         5: [('q', 1, 4), ('q', 1, 5), ('q', 1, 6),
                             ('a', 0, 0)],
                         6: [('q', 1, 7), ('q', 2, 0), ('q', 2, 1),
                             ('a', 0, 1), ('a', 0, 2)],
                         7: [('q', 2, 2), ('q', 2, 3), ('q', 2, 4),
                             ('a', 0, 3), ('a', 0, 4)]}
                for m in range(1, MT):
                    v_proj(m)
                    if m == 1:
                        w_o = stream_w(wo_d)
                    for kind, hh, j in sched.get(m, []):
                        if kind == 'q':
                            qk_exp_h(hh, j, dp0)
                        else:
                            av_sched(hh, j, opv, "oav", bufs=2)
                # boundary fill: av(0) tail + two more head-2 steps
                av_sched(0, 5, opv, "oav", bufs=2)
                qk_exp_h(2, 5, dp0)
                av_sched(0, 6, opv, "oav", bufs=2)
                qk_exp_h(2, 6, dp0)
                av_sched(0, 7, opv, "oav", bufs=2)
                dp0_cm.__exit__(None, None, None)
                opv_cm.__exit__(None, None, None)
                vpool_cm.__exit__(None, None, None)

            # ---------------- Phase C: attention ----------------
            if True:
                op_cm = tc.tile_pool(name="op", bufs=4, space="PSUM")
                op = op_cm.__enter__()
                dp_cm = tc.tile_pool(name="dp", bufs=2, space="PSUM")
                dp = dp_cm.__enter__()

                # AV for head h-1 rides along with QK/exp of head h (av(0)
                # already ran inside the V loop); the oa tag's 4 slots hold
                # two heads' accumulators (one accumulating, one finishing).
                for h in range(2, H):
                    for j in range(MT):
                        av_sched(h - 1, j, op, "oa")
                        if (h, j) not in es_store:
                            qk_exp_h(h, j, dp)
                # dots banks are dead after the last QK: close dp and give
                # the last head's AV fresh psum slots so the tail starts
                # without waiting on earlier eviction chains.
                dp_cm.__exit__(None, None, None)
                opx_cm = tc.tile_pool(name="opx", bufs=4, space="PSUM")
                opx = opx_cm.__enter__()
                for j in range(MT):
                    av_sched(H - 1, j, opx, "oax")
                opx_cm.__exit__(None, None, None)
                op_cm.__exit__(None, None, None)

            # ---------------- Phase D: Wo projection ----------------
            with (
                tc.tile_pool(name="fin", bufs=2) as fin,
                tc.tile_pool(name="fp", bufs=3, space="PSUM") as fp,
            ):
                for m in range(MT):
                    ps = fp.tile([128, DIM], F32, tag="fp", bufs=3)
                    for f in range(FT):
                        for nh in range(2):
                            sl = slice(nh * 512, (nh + 1) * 512)
                            nc.tensor.matmul(
                                ps[:, sl],
                                outT[:, f, m * 128:(m + 1) * 128],
                                w_o[:, f, nh * 512:(nh + 1) * 512],
                                start=(f == 0), stop=(f == FT - 1),
                            )
                    fs = fin.tile([128, DIM], F32, tag="fs", bufs=2)
                    for nh in range(2):
                        sl = slice(nh * 512, (nh + 1) * 512)
                        if nh == 0:
                            nc.scalar.copy(fs[:, sl], ps[:, sl])
                        else:
                            nc.vector.tensor_copy(fs[:, sl], ps[:, sl])
                    nc.sync.dma_start(out_d[m * 128:(m + 1) * 128, :], fs[:])

            cp_cm.__exit__(None, None, None)
            ep_cm.__exit__(None, None, None)

    nc.compile()
    return nc


import numpy as np
from concourse.bass_utils import run_bass_kernel_spmd

_NC_CACHE = {}


def _get_nc(has_bias: bool):
    if has_bias not in _NC_CACHE:
        _NC_CACHE[has_bias] = build_nc(has_bias)
    return _NC_CACHE[has_bias]


def _pmajor(w):
    """[DIM, C] -> [128, KT*C] partition-major relayout (contiguous DMA)."""
    d, c = w.shape
    kt = d // 128
    return np.ascontiguousarray(
        w.reshape(kt, 128, c).transpose(1, 0, 2).reshape(128, kt * c))


def host_prepare(x, mask, h_idx, w_idx, gamma_ln, beta_ln, q_gamma, k_gamma,
                 Wq, Wkv, Wo):
    x = np.asarray(x, np.float32)
    mask = np.asarray(mask)
    assert mask.all(), "kernel assumes all-True mask"
    assert np.allclose(np.asarray(q_gamma), 1.0), "kernel assumes q_gamma == 1"
    assert np.allclose(np.asarray(k_gamma), 1.0), "kernel assumes k_gamma == 1"

    gamma_ln = np.asarray(gamma_ln, np.float32)
    beta_ln = np.asarray(beta_ln, np.float32)
    Wq = np.asarray(Wq, np.float32)
    Wkv = np.asarray(Wkv, np.float32)
    Wo = np.asarray(Wo, np.float32)

    wq_f = _pmajor((gamma_ln[:, None] * Wq).astype(np.float16))
    wkv_g = (gamma_ln[:, None] * Wkv).astype(np.float16)
    wk_f = _pmajor(wkv_g[:, :INNER])
    wv_f = _pmajor(wkv_g[:, INNER:])
    wo_f = _pmajor(Wo.astype(np.float16))
    bq = (beta_ln @ Wq)[None, :].astype(np.float32)
    bkv = (beta_ln @ Wkv)[None, :].astype(np.float32)
    has_bias = bool(np.abs(bq).max() > 0 or np.abs(bkv).max() > 0)

    # RoPE tables; sqrt(DH)=8 and rotate-half signs folded in.
    h_idx = np.asarray(h_idx, np.float32)
    w_idx = np.asarray(w_idx, np.float32)
    dq = DH // 4
    inv_freq = 1.0 / (10000.0 ** (np.arange(dq, dtype=np.float32) / dq))
    th = h_idx[..., None] * inv_freq
    tw = w_idx[..., None] * inv_freq
    cos_t = (np.concatenate([np.cos(th), np.cos(th), np.cos(tw), np.cos(tw)], -1)
             * np.sqrt(np.float32(DH))).astype(np.float16)
    sin_full = (np.concatenate([np.sin(th), np.sin(th), np.sin(tw), np.sin(tw)], -1)
                * np.sqrt(np.float32(DH)))
    sign = np.tile(np.concatenate(
        [-np.ones(dq, np.float32), np.ones(dq, np.float32)]), 2)
    sin_t = (sin_full * sign).astype(np.float16)

    def trig_pmajor(t):
        # [N, DH] -> [128, MT*DH] partition-major
        return np.ascontiguousarray(
            t.reshape(MT, 128, DH).transpose(1, 0, 2).reshape(128, MT * DH))

    in_maps = []
    for b in range(B):
        m = {
            "x": np.ascontiguousarray(x[b]).astype(np.float16),
            "ident16": np.eye(128, dtype=np.float16),
            "wq": wq_f,
            "wk": wk_f,
            "wv": wv_f,
            "wo": wo_f,
            "cos_t": trig_pmajor(cos_t[b]),
            "sin_t": trig_pmajor(sin_t[b]),
        }
        if has_bias:
            m["bq"] = bq.astype(np.float16)
            m["bkv"] = bkv.astype(np.float16)
        in_maps.append(m)
    return in_maps, has_bias


def run(trace=False, **inputs):
    in_maps, has_bias = host_prepare(**inputs)
    nc = _get_nc(has_bias)
    res = run_bass_kernel_spmd(nc, in_maps, core_ids=list(range(B)), trace=trace)
    out = np.stack([res.results[c]["out"] for c in range(B)], axis=0)
    return out.astype(np.float32), res


def kernel(**inputs):
    out, _ = run(trace=False, **inputs)
    return out


if __name__ == "__main__":
    build_nc(False)
    print("build ok")



# revision 82
# speedup vs baseline: 1.0712x; 1.0087x over previous
"""Trainium2 Bass kernel for nn_Attention_22299470201527.

Dense transformer attention block:
  LayerNorm -> Wq/Wkv projections -> per-head QK RMSNorm -> 2D RoPE ->
  softmax(QK^T) V -> Wo projection,  B=8, N=1024, DIM=1024, H=16, DH=64.

Sharding: data-parallel over batch — 8 batch elements on 8 NeuronCores,
one per core, weights replicated, no collectives. kernel(**inputs) takes
the full unsharded inputs and returns the full [8, 1024, 1024] output.

Design notes (~313us HW exec; pair-granular predecessor 341us, f32r
baseline 397us):
  * All matmul operands fp16/bf16 (FWL weight loads, half the DMA bytes).
  * Weights/trig tables are relaid out host-side to partition-major so
    every DMA lands as 128 large contiguous descriptors.
  * x tiles are prefetched ahead of the weight streams; each weight
    matrix streams during the previous projection loop (wpool bufs=2).
  * LayerNorm is fused into the Q loop; LN math runs two iterations
    ahead, xn transposes one ahead of proj, and the RoPE-output PE
    transposes three behind, so the in-order PE queue rarely blocks on
    the eviction -> square -> reduce -> sqrt -> RoPE -> rinv chain.
  * Attention runs at HEAD granularity: one [128 keys, 1024 q] dots
    psum tile (2 banks) per (head, key-tile) step, double-buffered, so
    QK of step s+1 overlaps exp of step s and the ACT engine streams
    exp back-to-back at ~1.05us/tile — ACT is the phase-C roofline
    (128 exps ~ 138us).  PSUM: dots 2x2 banks + AV accumulators 4x1.
  * AV for head h-1 rides along with QK/exp of head h; the last head's
    AV tail runs on fresh psum banks (dots pool closed) to avoid WAR
    waits and a p-state dip.
  * Softmax denominators: the V stationary is [dh(64) | ones(64)], so
    AV psum rows 64-127 come out as 64 pre-broadcast copies of the
    denominator.  Normalization = one bf16 psum eviction (frees the
    bank fast) + reciprocal_approx_fast + multiply, all on DVE, with
    fp16 results written into the dead xnT buffer for Wo.  (No DRAM
    bounce, no partition broadcast: approx_fast only reads base-0
    SBUF — it silently returns garbage on PSUM or partition-64 APs.)
  * Heads 0-2 pre-compute QK/exp inside the V-projection loop (3 exp
    steps per 3.4us projection block keeps ACT fed), with av(0) riding
    the back half on its own psum bank pair (V projections run as
    1-bank halves to make room), so phase C starts with zero backlog.
  * Wo accumulates per token tile with a 3-deep psum pipeline; output
    eviction splits ACT/DVE halves, one 512KB DMA per tile.
"""

import sys

for _p in ("/opt/trn_rl_repo",):
    if _p not in sys.path:
        sys.path.append(_p)

import concourse.bacc as bacc
import concourse.bass as bass
import concourse.tile as tile
from concourse import mybir

F32 = mybir.dt.float32
F16 = mybir.dt.float16
BF16 = mybir.dt.bfloat16

B, N, DIM, H, DH = 8, 1024, 1024, 16, 64
INNER = H * DH
KT = DIM // 128
MT = N // 128
FT = INNER // 128
EPS_LN = 1e-5
EPS_NORM = 1e-12


def _bcast_heads(ap2d, nheads=H):
    """[128, D] AP -> [128, nheads, D], stride-0 broadcast over heads."""
    return bass.AP(
        tensor=ap2d.tensor, offset=ap2d.offset,
        ap=[ap2d.ap[0], [0, nheads], ap2d.ap[1]],
    )


def _bcast_last(ap2d, n):
    """[128, Hn] AP -> [128, Hn, n], stride-0 broadcast innermost."""
    return bass.AP(
        tensor=ap2d.tensor, offset=ap2d.offset,
        ap=[ap2d.ap[0], ap2d.ap[1], [0, n]],
    )


def _rot_view(tile_ap):
    """[128, 1024] tile viewed [128, H, 2, 2, 16] with adjacent 16-blocks
    swapped (rotate-half shuffle; signs live in the sin table)."""
    return bass.AP(
        tensor=tile_ap.tensor, offset=tile_ap.offset + 16,
        ap=[tile_ap.ap[0], [DH, H], [32, 2], [-16, 2], [1, 16]],
    )


def build_nc(has_bias: bool):
    nc = bacc.Bacc("TRN2", target_bir_lowering=False, debug=False, num_devices=8)

    x_d = nc.dram_tensor("x", [N, DIM], F32, kind="ExternalInput")
    # weights partition-major: [128, KT, INNER] flattened per partition
    wq_d = nc.dram_tensor("wq", [128, KT * INNER], F16, kind="ExternalInput")
    wk_d = nc.dram_tensor("wk", [128, KT * INNER], F16, kind="ExternalInput")
    wv_d = nc.dram_tensor("wv", [128, KT * INNER], F16, kind="ExternalInput")
    wo_d = nc.dram_tensor("wo", [128, KT * INNER], F16, kind="ExternalInput")
    id16_d = nc.dram_tensor("ident16", [128, 128], F16, kind="ExternalInput")
    cos_d = nc.dram_tensor("cos_t", [128, MT * DH], F16, kind="ExternalInput")
    sin_d = nc.dram_tensor("sin_t", [128, MT * DH], F16, kind="ExternalInput")
    if has_bias:
        bq_d = nc.dram_tensor("bq", [1, INNER], F16, kind="ExternalInput")
        bkv_d = nc.dram_tensor("bkv", [1, 2 * INNER], F16, kind="ExternalInput")
    out_d = nc.dram_tensor("out", [N, DIM], F32, kind="ExternalOutput")

    with tile.TileContext(nc) as tc:
        with (
            tc.tile_pool(name="const", bufs=1) as constp,
            tc.tile_pool(name="wpool", bufs=2) as wpool,
            tc.tile_pool(name="stats", bufs=2) as stats,
            tc.tile_pool(name="bc", bufs=1) as bc,
            tc.tile_pool(name="xa", bufs=1) as xa,
        ):
            # Prefetch first x tiles BEFORE everything else so LayerNorm
            # starts immediately.
            XPRE = 2
            x_tiles = {}

            def x_fetch(m):
                x_t = xa.tile([128, DIM], F32, tag="x", bufs=2, name=f"x{m%2}")
                nc.sync.dma_start(x_t[:], x_d[m * 128:(m + 1) * 128, :])
                x_tiles[m] = x_t

            for m in range(XPRE):
                x_fetch(m)

            ident_h = constp.tile([128, 128], F16)
            nc.sync.dma_start(ident_h[:], id16_d[:])
            eps_t = constp.tile([128, 1], F32)
            nc.vector.memset(eps_t[:], EPS_LN)

            def stream_w(dram_t):
                w = wpool.tile([128, KT, INNER], F16, tag="w")
                nc.sync.dma_start(
                    w[:], dram_t[:].rearrange("p (a i) -> p a i", a=KT)
                )
                return w

            w_q = stream_w(wq_d)

            cos_sb = constp.tile([128, MT, DH], F16)
            sin_sb = constp.tile([128, MT, DH], F16)
            nc.sync.dma_start(cos_sb[:], cos_d[:].rearrange("p (a d) -> p a d", a=MT))
            nc.sync.dma_start(sin_sb[:], sin_d[:].rearrange("p (a d) -> p a d", a=MT))
            bq_sb = bkv_sb = ones1 = None
            if has_bias:
                bq_sb = constp.tile([1, INNER], F16)
                bkv_sb = constp.tile([1, 2 * INNER], F16)
                nc.sync.dma_start(bq_sb[:], bq_d[:])
                nc.sync.dma_start(bkv_sb[:], bkv_d[:])
                ones1 = constp.tile([1, 128], F16)
                nc.vector.memset(ones1[:], 1.0)

            # Long-lived activations.
            qT = bc.tile([128, FT, N], F16)
            kT = bc.tile([128, FT, N], F16)
            xnT = bc.tile([128, KT, N], F16)   # reused as outT in phase C/D
            # V stationary is [dh (64) | ones (64)] per head: AV psum rows
            # 64-127 come out as 64 pre-broadcast copies of the softmax
            # denominator, so normalization needs no partition broadcast.
            v_sb = bc.tile([128, MT, H, 2 * DH], BF16)
            nc.gpsimd.memset(
                bass.AP(
                    tensor=v_sb.tensor, offset=v_sb[:].offset + DH,
                    ap=[v_sb[:].ap[0], [H * 2 * DH, MT], [2 * DH, H],
                        [1, DH]],
                ),
                1.0,
            )

            ep_cm = tc.tile_pool(name="ep", bufs=1)
            ep = ep_cm.__enter__()
            cp_cm = tc.tile_pool(name="cpool", bufs=1)
            cpool = cp_cm.__enter__()

            with tc.tile_pool(name="pb", bufs=1) as pb:
                pp_cm = tc.tile_pool(name="pp", bufs=2, space="PSUM")
                pp = pp_cm.__enter__()
                tp_cm = tc.tile_pool(name="tp", bufs=4, space="PSUM")
                tp = tp_cm.__enter__()
                def ln_math(m):
                    """LayerNorm stats + normalized fp16 tile for x tile m."""
                    x_t = x_tiles.pop(m)
                    st = stats.tile([128, 2, 6], F32, tag="bst")
                    for g in range(2):
                        nc.vector.bn_stats(st[:, g, :], x_t[:, g * 512:(g + 1) * 512])
                    mv = stats.tile([128, 2], F32, tag="mv")
                    nc.vector.bn_aggr(mv[:], st[:])
                    sd = stats.tile([128, 1], F32, tag="sd")
                    nc.scalar.activation(
                        sd[:], mv[:, 1:2], mybir.ActivationFunctionType.Sqrt,
                        bias=eps_t[:], scale=1.0,
                    )
                    rstd = stats.tile([128, 1], F32, tag="rstd")
                    nc.vector.reciprocal(rstd[:], sd[:])
                    nmu = stats.tile([128, 1], F32, tag="nmu")
                    nc.vector.scalar_tensor_tensor(
                        out=nmu[:], in0=mv[:, 0:1], scalar=-1.0, in1=rstd[:],
                        op0=mybir.AluOpType.mult, op1=mybir.AluOpType.mult,
                    )
                    xn_t = xa.tile([128, DIM], F16, tag="xn", bufs=3)
                    nc.scalar.activation(
                        xn_t[:], x_t[:], mybir.ActivationFunctionType.Identity,
                        bias=nmu[:], scale=rstd[:],
                    )
                    if m + XPRE < MT:
                        x_fetch(m + XPRE)
                    return xn_t

                def ln_transp(xn_t, m):
                    for g in range(2):
                        tps = tp.tile([128, 512], F16, tag="tp", bufs=4)
                        for b4 in range(4):
                            k = g * 4 + b4
                            nc.tensor.transpose(
                                tps[:, b4 * 128:(b4 + 1) * 128],
                                xn_t[:, k * 128:(k + 1) * 128],
                                ident_h[:],
                            )
                        # split the two evictions across ACT and DVE so the
                        # dependent proj(m) isn't gated by the ACT queue
                        dst = xnT[:, g * 4:(g + 1) * 4, m * 128:(m + 1) * 128]
                        src = tps[:].rearrange("p (a t) -> p a t", a=4)
                        if g == 0:
                            nc.scalar.copy(dst, src)
                        else:
                            nc.vector.tensor_copy(dst, src)

                def proj(w, m, psp, bias_sb=None, bias_off=0):
                    """One [128, INNER] projection psum tile for token tile m.
                    Matmul moving free dim is capped at 512, so each half is
                    its own accumulation group."""
                    ps = psp.tile([128, INNER], F32, tag="pp", bufs=2)
                    for nh in range(2):
                        sl = slice(nh * 512, (nh + 1) * 512)
                        if bias_sb is not None:
                            nc.tensor.matmul(
                                ps[:, sl], ones1[:],
                                bias_sb[:, bias_off + nh * 512:
                                        bias_off + (nh + 1) * 512],
                                start=True, stop=False,
                            )
                        for k in range(KT):
                            nc.tensor.matmul(
                                ps[:, sl],
                                xnT[:, k, m * 128:(m + 1) * 128],
                                w[:, k, nh * 512:(nh + 1) * 512],
                                start=(k == 0 and bias_sb is None),
                                stop=(k == KT - 1),
                            )
                    return ps

                def rms_rope(ps, m, dve_add=False):
                    qtmp = pb.tile([128, INNER], F16, tag="qtmp", bufs=2)
                    nc.scalar.copy(qtmp[:], ps[:])
                    sq = pb.tile([128, INNER], F16, tag="sq", bufs=1)
                    nc.scalar.activation(
                        sq[:], qtmp[:], mybir.ActivationFunctionType.Square,
                        bias=0.0, scale=1.0,
                    )
                    ssq = stats.tile([128, H], F32, tag="ssq")
                    nc.vector.reduce_sum(
                        ssq[:], sq[:].rearrange("p (h d) -> p h d", h=H),
                        axis=mybir.AxisListType.X,
                    )
                    nrm = stats.tile([128, H], F32, tag="nrm")
                    nc.scalar.activation(
                        nrm[:], ssq[:], mybir.ActivationFunctionType.Sqrt,
                        bias=0.0, scale=1.0,
                    )
                    rinv = stats.tile([128, H], F32, tag="rinv")
                    nc.vector.reciprocal(rinv[:], nrm[:])

                    q3 = qtmp[:].rearrange("p (h d) -> p h d", h=H)
                    t1 = pb.tile([128, INNER], F16, tag="t1", bufs=2)
                    nc.vector.tensor_mul(
                        t1[:].rearrange("p (h d) -> p h d", h=H),
                        q3, _bcast_heads(cos_sb[:, m, :]),
                    )
                    t2 = pb.tile([128, INNER], F16, tag="t2", bufs=2)
                    sin_b = bass.AP(
                        tensor=sin_sb.tensor,
                        offset=sin_sb[:, m, :].offset,
                        ap=[sin_sb[:, m, :].ap[0], [0, H], [32, 2], [16, 2],
                            [1, 16]],
                    )
                    nc.vector.tensor_mul(
                        t2[:].rearrange("p (h a b c) -> p h a b c",
                                        h=H, a=2, b=2, c=16),
                        _rot_view(qtmp[:]), sin_b,
                    )
                    if dve_add:
                        # split the rope add across DVE and Pool so the Pool
                        # engine (2x slower per element) stops pacing the loop
                        nc.vector.tensor_add(t1[:, 0:512], t1[:, 0:512], t2[:, 0:512])
                        nc.gpsimd.tensor_add(t1[:, 512:1024], t1[:, 512:1024], t2[:, 512:1024])
                    else:
                        nc.gpsimd.tensor_add(t1[:], t1[:], t2[:])
                    qr = pb.tile([128, INNER], F16, tag="qr", bufs=4)
                    nc.gpsimd.tensor_mul(
                        qr[:].rearrange("p (h d) -> p h d", h=H),
                        t1[:].rearrange("p (h d) -> p h d", h=H),
                        _bcast_last(rinv[:], DH),
                    )
                    return qr

                def transp(qr, m, dst):
                    for g in range(2):
                        tps = tp.tile([128, 512], F16, tag="tp", bufs=4)
                        for b4 in range(4):
                            f = g * 4 + b4
                            nc.tensor.transpose(
                                tps[:, b4 * 128:(b4 + 1) * 128],
                                qr[:, f * 128:(f + 1) * 128],
                                ident_h[:],
                            )
                        nc.scalar.copy(
                            dst[:, g * 4:(g + 1) * 4, m * 128:(m + 1) * 128],
                            tps[:].rearrange("p (a t) -> p a t", a=4),
                        )

                # ---- Q loop (LayerNorm fused; LN math runs two iterations
                # ahead and RoPE-output transposes two behind so the PE
                # in-order queue never waits on the ACT/DVE/GPS chains) ----
                xns = {0: ln_math(0), 1: ln_math(1)}
                # rope outputs pending PE transposition; the 3-deep queue
                # carries across the Q->K boundary so the K projections fill
                # the wait on Q's last rope chains instead of the PE idling
                pending = []
                w_k = None
                # transpose m+1's xn tile while proj(m) runs so proj never
                # heads-of-line blocks on its own xnT eviction
                ln_transp(xns.pop(0), 0)
                for m in range(MT):
                    if m + 1 < MT:
                        ln_transp(xns.pop(m + 1), m + 1)
                    ps = proj(w_q, m, pp, bq_sb, 0)
                    if m == 0:
                        w_k = stream_w(wk_d)   # lands during the q loop
                    if len(pending) >= 3:
                        transp(*pending.pop(0))
                    pending.append((rms_rope(ps, m), m, qT))
                    if m + 2 < MT:
                        xns[m + 2] = ln_math(m + 2)

                # ---- K loop ----
                w_v = None
                for m in range(MT):
                    ps = proj(w_k, m, pp, bkv_sb, 0)
                    if m == 0:
                        w_v = stream_w(wv_d)
                    if len(pending) >= 3:
                        transp(*pending.pop(0))
                    pending.append((rms_rope(ps, m, dve_add=True), m, kT))
                es_store = {}

                def qk_exp_h(h, j, dpool):
                    """QK + exp for head h, key tile j. One [128, N] dots
                    tile (2 banks) per step, double-buffered: QK(s+1) runs
                    during exp(s), so exp never waits on the PE."""
                    f, pb_ = h >> 1, (h & 1) * 64
                    dots = dpool.tile([128, N], F32, tag="dots", bufs=2)
                    for qh in range(2):
                        qsl = slice(qh * 512, (qh + 1) * 512)
                        nc.tensor.matmul(
                            dots[:, qsl],
                            kT[pb_:pb_ + 64, f, j * 128:(j + 1) * 128],
                            qT[pb_:pb_ + 64, f, qsl],
                            start=True, stop=True,
                        )
                    e_t = ep.tile([128, N], BF16, tag="E", bufs=22)
                    nc.scalar.activation(
                        e_t[:], dots[:], mybir.ActivationFunctionType.Exp)
                    es_store[(h, j)] = e_t

                outT = xnT  # xnT dead after projections; reuse for attn out

                def av_alloc_h(pool, tag, bufs=4):
                    return [pool.tile([128, 512], F32, tag=tag, bufs=bufs,
                                      name=f"oa{qh}") for qh in range(2)]

                def av_step_h(h, j, oas2):
                    for qh in range(2):
                        nc.tensor.matmul(
                            oas2[qh], v_sb[:, j, h, :],
                            es_store[(h, j)][:, qh * 512:(qh + 1) * 512],
                            start=(j == 0), stop=(j == MT - 1),
                        )

                def av_finish_h(h, oas2):
                    """Softmax-normalize: psum rows 64-127 hold 64 identical
                    copies of the denominator (ones half of the V stationary);
                    recip them on DVE and fuse the divide into the psum
                    eviction, writing fp16 straight into outT."""
                    f, pb_ = h >> 1, (h & 1) * 64
                    scs = []
                    for qh in range(2):
                        sc = cpool.tile([128, 512], BF16, tag="sc", bufs=4)
                        nc.vector.tensor_copy(sc[:], oas2[qh][:])
                        scs.append(sc)
                    for qh, sc in enumerate(scs):
                        dn = cpool.tile([64, 512], F32, tag="dn", bufs=2)
                        nc.vector.tensor_copy(dn[:], sc[64:128, :])
                        rq = cpool.tile([64, 512], F32, tag="rq", bufs=2)
                        nc.vector.reciprocal_approx_fast(rq[:], dn[:])
                        nc.vector.tensor_mul(
                            outT[pb_:pb_ + 64, f, qh * 512:(qh + 1) * 512],
                            sc[0:DH, :], rq[:],
                        )

                oas_map = {}

                def av_sched(h_av, j, pool, tag, bufs=4):
                    if j == 0:
                        oas_map[h_av] = av_alloc_h(pool, tag, bufs)
                    av_step_h(h_av, j, oas_map[h_av])
                    if j == MT - 1:
                        av_finish_h(h_av, oas_map.pop(h_av))

                def v_proj(m):
                    for nh in range(2):
                        ps = vpool.tile([128, 512], F32, tag="vp", bufs=2,
                                        name=f"vp{nh}")
                        sl = slice(nh * 512, (nh + 1) * 512)
                        if bkv_sb is not None:
                            nc.tensor.matmul(
                                ps[:], ones1[:],
                                bkv_sb[:, INNER + nh * 512:
                                       INNER + (nh + 1) * 512],
                                start=True, stop=False,
                            )
                        for k in range(KT):
                            nc.tensor.matmul(
                                ps[:], xnT[:, k, m * 128:(m + 1) * 128],
                                w_v[:, k, sl],
                                start=(k == 0 and bkv_sb is None),
                                stop=(k == KT - 1),
                            )
                        nc.vector.tensor_copy(
                            v_sb[:, m, nh * 8:(nh + 1) * 8, 0:DH],
                            ps[:].rearrange("p (h d) -> p h d", h=8),
                        )

                # ---- k tail + V loop with heads 0-2 QK/exp and av(0) ----
                # All three k-tail transposes drain, then the A/B psum pools
                # close so the V projections (1-bank halves), pair-0 dots and
                # av(0)'s accumulators fit in 8 banks together.
                transp(*pending.pop(0))
                transp(*pending.pop(0))
                transp(*pending.pop(0))
                tp_cm.__exit__(None, None, None)
                pp_cm.__exit__(None, None, None)
                vpool_cm = tc.tile_pool(name="vpool", bufs=2, space="PSUM")
                vpool = vpool_cm.__enter__()
                opv_cm = tc.tile_pool(name="opv", bufs=2, space="PSUM")
                opv = opv_cm.__enter__()
                dp0_cm = tc.tile_pool(name="dp0", bufs=2, space="PSUM")
                dp0 = dp0_cm.__enter__()
                v_proj(0)

                w_o = None
                # 3 QK/exp steps per v_proj block so ACT (~3.3us of exp)
                # stays fed through each 3.4us projection; av(0) rides the
                # back half once its es tiles and v_sb key-tiles exist.
                sched = {1: [('q', 0, 0), ('q', 0, 1), ('q', 0, 2)],
                         2: [('q', 0, 3), ('q', 0, 4), ('q', 0, 5)],
                         3: [('q', 0, 6), ('q', 0, 7), ('q', 1, 0)],
                         4: [('q', 1, 1), ('q', 1, 2), ('q', 1, 3)],
                         5: [('q', 1, 4), ('q', 1, 5), ('q', 1, 6),
                             ('a', 0, 0)],
                         6: [('q', 1, 7), ('q', 2, 0), ('q', 2, 1),
                             ('a', 0, 1), ('a', 0, 2)],
                         7: [('q', 2, 2), ('q', 2, 3), ('q', 2, 4),
                             ('a', 0, 3), ('a', 0, 4)]}
                for m in range(1, MT):
                    v_proj(m)
                    if m == 1:
                        w_o = stream_w(wo_d)
                    for kind, hh, j in sched.get(m, []):
                        if kind == 'q':
                            qk_exp_h(hh, j, dp0)
                        else:
                            av_sched(hh, j, opv, "oav", bufs=2)
                # boundary fill: av(0) tail
                av_sched(0, 5, opv, "oav", bufs=2)
                av_sched(0, 6, opv, "oav", bufs=2)
                av_sched(0, 7, opv, "oav", bufs=2)
                dp0_cm.__exit__(None, None, None)
                opv_cm.__exit__(None, None, None)
                vpool_cm.__exit__(None, None, None)

            # ---------------- Phase C: attention ----------------
            if True:
                op_cm = tc.tile_pool(name="op", bufs=4, space="PSUM")
                op = op_cm.__enter__()
                dp_cm = tc.tile_pool(name="dp", bufs=2, space="PSUM")
                dp = dp_cm.__enter__()

                # AV for head h-1 rides along with QK/exp of head h (av(0)
                # already ran inside the V loop); the oa tag's 4 slots hold
                # two heads' accumulators (one accumulating, one finishing).
                for h in range(2, H):
                    for j in range(MT):
                        av_sched(h - 1, j, op, "oa")
                        if (h, j) not in es_store:
                            qk_exp_h(h, j, dp)
                # dots banks are dead after the last QK: close dp and give
                # the last head's AV fresh psum slots so the tail starts
                # without waiting on earlier eviction chains.
                dp_cm.__exit__(None, None, None)
                opx_cm = tc.tile_pool(name="opx", bufs=4, space="PSUM")
                opx = opx_cm.__enter__()
                for j in range(MT):
                    av_sched(H - 1, j, opx, "oax")
                opx_cm.__exit__(None, None, None)
                op_cm.__exit__(None, None, None)

            # ---------------- Phase D: Wo projection ----------------
            with (
                tc.tile_pool(name="fin", bufs=2) as fin,
                tc.tile_pool(name="fp", bufs=3, space="PSUM") as fp,
            ):
                for m in range(MT):
                    ps = fp.tile([128, DIM], F32, tag="fp", bufs=3)
                    for f in range(FT):
                        for nh in range(2):
                            sl = slice(nh * 512, (nh + 1) * 512)
                            nc.tensor.matmul(
                                ps[:, sl],
                                outT[:, f, m * 128:(m + 1) * 128],
                                w_o[:, f, nh * 512:(nh + 1) * 512],
                                start=(f == 0), stop=(f == FT - 1),
                            )
                    fs = fin.tile([128, DIM], F32, tag="fs", bufs=2)
                    for nh in range(2):
                        sl = slice(nh * 512, (nh + 1) * 512)
                        if nh == 0:
                            nc.scalar.copy(fs[:, sl], ps[:, sl])
                        else:
                            nc.vector.tensor_copy(fs[:, sl], ps[:, sl])
                    nc.sync.dma_start(out_d[m * 128:(m + 1) * 128, :], fs[:])

            cp_cm.__exit__(None, None, None)
            ep_cm.__exit__(None, None, None)

    nc.compile()
    return nc


import numpy as np
from concourse.bass_utils import run_bass_kernel_spmd

_NC_CACHE = {}


def _get_nc(has_bias: bool):
    if has_bias not in _NC_CACHE:
        _NC_CACHE[has_bias] = build_nc(has_bias)
    return _NC_CACHE[has_bias]


def _pmajor(w):
    """[DIM, C] -> [128, KT*C] partition-major relayout (contiguous DMA)."""
    d, c = w.shape
    kt = d // 128
    return np.ascontiguousarray(
        w.reshape(kt, 128, c).transpose(1, 0, 2).reshape(128, kt * c))


def host_prepare(x, mask, h_idx, w_idx, gamma_ln, beta_ln, q_gamma, k_gamma,
                 Wq, Wkv, Wo):
    x = np.asarray(x, np.float32)
    mask = np.asarray(mask)
    assert mask.all(), "kernel assumes all-True mask"
    assert np.allclose(np.asarray(q_gamma), 1.0), "kernel assumes q_gamma == 1"
    assert np.allclose(np.asarray(k_gamma), 1.0), "kernel assumes k_gamma == 1"

    gamma_ln = np.asarray(gamma_ln, np.float32)
    beta_ln = np.asarray(beta_ln, np.float32)
    Wq = np.asarray(Wq, np.float32)
    Wkv = np.asarray(Wkv, np.float32)
    Wo = np.asarray(Wo, np.float32)

    wq_f = _pmajor((gamma_ln[:, None] * Wq).astype(np.float16))
    wkv_g = (gamma_ln[:, None] * Wkv).astype(np.float16)
    wk_f = _pmajor(wkv_g[:, :INNER])
    wv_f = _pmajor(wkv_g[:, INNER:])
    wo_f = _pmajor(Wo.astype(np.float16))
    bq = (beta_ln @ Wq)[None, :].astype(np.float32)
    bkv = (beta_ln @ Wkv)[None, :].astype(np.float32)
    has_bias = bool(np.abs(bq).max() > 0 or np.abs(bkv).max() > 0)

    # RoPE tables; sqrt(DH)=8 and rotate-half signs folded in.
    h_idx = np.asarray(h_idx, np.float32)
    w_idx = np.asarray(w_idx, np.float32)
    dq = DH // 4
    inv_freq = 1.0 / (10000.0 ** (np.arange(dq, dtype=np.float32) / dq))
    th = h_idx[..., None] * inv_freq
    tw = w_idx[..., None] * inv_freq
    cos_t = (np.concatenate([np.cos(th), np.cos(th), np.cos(tw), np.cos(tw)], -1)
             * np.sqrt(np.float32(DH))).astype(np.float16)
    sin_full = (np.concatenate([np.sin(th), np.sin(th), np.sin(tw), np.sin(tw)], -1)
                * np.sqrt(np.float32(DH)))
    sign = np.tile(np.concatenate(
        [-np.ones(dq, np.float32), np.ones(dq, np.float32)]), 2)
    sin_t = (sin_full * sign).astype(np.float16)

    def trig_pmajor(t):
        # [N, DH] -> [128, MT*DH] partition-major
        return np.ascontiguousarray(
            t.reshape(MT, 128, DH).transpose(1, 0, 2).reshape(128, MT * DH))

    in_maps = []
    for b in range(B):
        m = {
            "x": np.ascontiguousarray(x[b]),
            "ident16": np.eye(128, dtype=np.float16),
            "wq": wq_f,
            "wk": wk_f,
            "wv": wv_f,
            "wo": wo_f,
            "cos_t": trig_pmajor(cos_t[b]),
            "sin_t": trig_pmajor(sin_t[b]),
        }
        if has_bias:
            m["bq"] = bq.astype(np.float16)
            m["bkv"] = bkv.astype(np.float16)
        in_maps.append(m)
    return in_maps, has_bias


def run(trace=False, **inputs):
    in_maps, has_bias = host_prepare(**inputs)
    nc = _get_nc(has_bias)
    res = run_bass_kernel_spmd(nc, in_maps, core_ids=list(range(B)), trace=trace)
    out = np.stack([res.results[c]["out"] for c in range(B)], axis=0)
    return out.astype(np.float32), res


def kernel(**inputs):
    out, _ = run(trace=False, **inputs)
    return out


if __name__ == "__main__":
    build_nc(False)
    print("build ok")

